# revision 26
# baseline (speedup 1.0000x reference)
"""GQA attention kernel for 8 TRN2 NeuronCores.

Problem: B=2, N=2048, DIM=1024, 16 q-heads / 4 kv-heads, head dim 64.
Sharding: core c handles batch c//4 and kv-head group c%4 (4 q-heads sharing
one kv head).  Wq/Wk/Wv column-sharded, Wo row-sharded; the Wo row reduction
(4 cores per batch) and the bias add happen on the host.

Per-core algorithm:
  KT = Wk_dup.T @ x.T            [128, 2048]   (kv head duplicated twice)
  QT = Wq_shard.T @ x.T          [256, 2048]   (4 heads stacked as 2x128)
  V  = x @ Wv_shard              [2048, 64]  (bf16, key chunks on partitions)
  per head h, 1024-wide q block qc:
    for each 128-key chunk kc:
      ss[k, q]  = K.Q^T           (bf16 matmuls, psum [128, 1024])
                + mask            (fp8 DoubleRow matmul: -240 on masked
                                   entries; exp(scale*(s-240)) ~ 0)
      ee = exp(ss/8)  (Act engine, bf16)
      O[q, d]  += ee_chunk.T @ V_chunk   (8 matmuls out [128, 64], cheap
                                          free dim = 64 instead of 512)
      den[q]   += ee_chunk.T @ ones      (1-row matmuls into D columns)
    O and den live in one psum slot: bank X = O [128, 8, 64],
    bank X+1 = D [128, 8].
    normalize: rcp = 1/D (DVE), On2[:, h, :] = O * rcp (per-partition scalar)
  per (qc, m=head pair, qch): PE-transpose On2 [128(q), 2x64] -> OT
    [128(h,d), 128(q)] (bf16 psum), DVE copy into OTn[m].
  out rows t: pf = OTn[0].T_t @ Wo_pair0 + OTn[1].T_t @ Wo_pair1 (psum),
    DVE copy -> bf16, DMA out.

Emission is software-pipelined: per kc we emit scores(kc), exp(kc), then
PV(kc-2), so the PE never waits on the Activation engine.  Block
post-processing (normalize / transpose / projection) is interleaved into
subsequent blocks' key loops.
"""

import os
import sys

for _p in ("/opt/trn_rl_repo",):
    if _p not in sys.path:
        sys.path.insert(0, _p)

import numpy as np
import ml_dtypes

import concourse.bass as bass  # noqa: F401  (registers AP machinery)
import concourse.tile as tile
from concourse import bacc, mybir
from concourse.bass_utils import run_bass_kernel_spmd

F32 = mybir.dt.float32
F8 = mybir.dt.float8e4
BF16 = mybir.dt.bfloat16
EXP = mybir.ActivationFunctionType.Exp
DRMODE = mybir.MatmulPerfMode.DoubleRow

B, NTOK, DIM = 2, 2048, 1024
H, KVH, DH = 16, 4, 64
P = 128
TQ = 1024  # q-block width for the attention inner loop
SCALE = DH ** -0.5

N_CORES = 8
# bisect stages: 1=projections only, 2=+scores/mask/exp, 3=+PV/den,
# 4=+norm, 5=full (default)
KSTAGE = int(os.environ.get("KSTAGE", "5"))
# mask variants: dr (DoubleRow fp8), f8 (standard fp8 matmul), none
KMASK = os.environ.get("KMASK", "dr")
# score variants: dr (DoubleRow fp8 K/Q), bf16 (via QT4)
KSCORE = os.environ.get("KSCORE", "bf16")


def _build_kernel():
    nc = bacc.Bacc("TRN2", target_bir_lowering=False, debug=False,
                   num_devices=N_CORES)

    xT_d = nc.dram_tensor("xT", [DIM, NTOK], BF16, kind="ExternalInput")
    km_d = nc.dram_tensor("keepM2", [NTOK // 2, 2, NTOK], F8,
                          kind="ExternalInput")
    i2_d = nc.dram_tensor("ident2", [64, 2, P], F8, kind="ExternalInput")
    im_d = nc.dram_tensor("identM", [P, P], F8, kind="ExternalInput")
    it_d = nc.dram_tensor("identT", [P, P], BF16, kind="ExternalInput")
    wq_d = nc.dram_tensor("wq", [DIM, 256], BF16, kind="ExternalInput")
    wk_d = nc.dram_tensor("wk2", [DIM, 128], BF16, kind="ExternalInput")
    wv_d = nc.dram_tensor("wv", [DIM, DH], BF16, kind="ExternalInput")
    wo_d = nc.dram_tensor("wo", [256, DIM], BF16, kind="ExternalInput")
    out_d = nc.dram_tensor("out", [NTOK, DIM], BF16, kind="ExternalOutput")

    with tile.TileContext(nc) as tc:
        with tc.tile_pool(name="persist", bufs=1) as pp, \
             tc.tile_pool(name="work", bufs=3) as wp, \
             tc.tile_pool(name="otnp", bufs=2) as op_, \
             tc.tile_pool(name="psS", bufs=3, space="PSUM") as psS, \
             tc.tile_pool(name="psO", bufs=2, space="PSUM") as psO:
            # ---- resident tensors -------------------------------------
            # head is DMA-bound: spread the first xT blocks over all four
            # DMA-capable queues, weights needed first go first.
            wk = pp.tile([P, 8, P], BF16, tag="wk")
            nc.scalar.dma_start(wk[:], wk_d.ap().rearrange("(o p) m -> p o m", p=P))
            xT = pp.tile([P, 8, NTOK], BF16, tag="xT")
            # 512KB grouped transfers: DMA dispatch is ~625ns/transfer, so
            # fewer, bigger chunks land the first q-block much sooner.
            xT_view = xT_d.ap().rearrange("(o p) m -> p o m", p=P)
            for nq in range(2):
                for og in range(2):
                    eng = nc.sync if og == 0 else nc.gpsimd
                    eng.dma_start(
                        xT[:, og * 4:(og + 1) * 4, nq * 512:(nq + 1) * 512],
                        xT_view[:, og * 4:(og + 1) * 4,
                                nq * 512:(nq + 1) * 512])
            wq = pp.tile([P, 8, 256], BF16, tag="wq")
            nc.sync.dma_start(wq[:], wq_d.ap().rearrange("(o p) m -> p o m", p=P))
            ident2 = pp.tile([64, 2, P], F8, tag="ident2")
            nc.scalar.dma_start(ident2[:], i2_d[:, :, :])
            identM = pp.tile([P, P], F8, tag="identM")
            nc.scalar.dma_start(identM[:], im_d[:, :])
            for nq in range(2, 4):
                for og in range(2):
                    eng = nc.sync if og == 0 else nc.gpsimd
                    eng.dma_start(
                        xT[:, og * 4:(og + 1) * 4, nq * 512:(nq + 1) * 512],
                        xT_view[:, og * 4:(og + 1) * 4,
                                nq * 512:(nq + 1) * 512])
            identT = pp.tile([P, P], BF16, tag="identT")
            nc.scalar.dma_start(identT[:], it_d[:, :])
            wv = pp.tile([P, 8, DH], BF16, tag="wv")
            nc.scalar.dma_start(wv[:], wv_d.ap().rearrange("(o p) m -> p o m", p=P))
            wo2 = pp.tile([P, 2, DIM], BF16, tag="wo2")
            for mm in range(2):
                nc.scalar.dma_start(wo2[:, mm, :],
                                    wo_d[mm * P:(mm + 1) * P, :])
            ones = pp.tile([P, 1], BF16, tag="ones")
            nc.vector.memset(ones[:], 1.0)

            # prime the PE p-state ramp while the first DMAs land: dummy
            # matmuls on a memset tile keep the array busy from t~0.2us so
            # the first real projections run at full clock
            dm = pp.tile([P, 512], BF16, tag="dm")
            nc.vector.memset(dm[:], 0.0)
            for i in range(12):
                dps = psS.tile([P, 512], F32, tag="s", name=f"dummy{i}")
                nc.tensor.matmul(dps[:], lhsT=dm[:, 0:P], rhs=dm[:, :],
                                 start=True, stop=True)

            # ---- projections ------------------------------------------
            KT = pp.tile([P, NTOK], BF16, tag="KT")

            def emit_kt(n):
                ps = psS.tile([P, 512], F32, tag="s", name=f"kt{n}")
                for d in range(8):
                    nc.tensor.matmul(ps[:], lhsT=(wk[:, d, :]),
                                     rhs=(xT[:, d, n * 512:(n + 1) * 512]),
                                     start=(d == 0), stop=(d == 7))
                nc.vector.tensor_copy(out=KT[:, n * 512:(n + 1) * 512], in_=ps[:])
                if KSCORE == "dr":
                    emit_kt8(n)

            QT = pp.tile([P, 2, NTOK], BF16, tag="QT")
            # duplicated-head layout: head hh twice along partitions, so score
            # matmuls use full 128-row tiles (DR matmuls break on HW when
            # mixed with 64-row tiles at partition 64 + psum slot reuse)
            QT4 = pp.tile([P, 4, NTOK], BF16, tag="QT4")
            # fp8 DoubleRow layouts: contraction (p, i) with the head dim
            # replicated in the i slot (scores come out doubled, same as the
            # bf16 dup path; exp scale folds the factor away).  fp8
            # conversions run on the otherwise-idle Pool engine (SBUF only).
            KT8 = pp.tile([64, 2, NTOK], F8, tag="KT8")
            QT8 = pp.tile([64, 4, 2, NTOK], F8, tag="QT8")
            QTH = pp.tile([64, 2, NTOK], BF16, tag="QTH")

            def emit_kt8(n):
                for i in range(2):
                    nc.gpsimd.tensor_copy(
                        out=KT8[:, i, n * 512:(n + 1) * 512],
                        in_=KT[0:64, n * 512:(n + 1) * 512])

            def emit_qt8(m, h, n):
                hh = 2 * m + h
                sl = slice(n * 512, (n + 1) * 512)
                if h == 0:
                    srcq = QT[0:64, m, sl]
                else:
                    nc.sync.dma_start(QTH[:, m, sl], QT[64:128, m, sl])
                    srcq = QTH[:, m, sl]
                for i in range(2):
                    nc.gpsimd.tensor_copy(out=QT8[:, hh, i, sl], in_=srcq)

            def emit_qt4(m, h, n):
                if KSCORE == "dr":
                    emit_qt8(m, h, n)
                    return
                hh = 2 * m + h
                src_ap = QT[h * DH:(h + 1) * DH, m, n * 512:(n + 1) * 512]
                eng = nc.gpsimd if (n + h) % 2 else nc.sync
                eng.dma_start(QT4[0:64, hh, n * 512:(n + 1) * 512], src_ap)
                eng.dma_start(QT4[64:128, hh, n * 512:(n + 1) * 512], src_ap)

            def emit_qt(m, n):
                ps = psS.tile([P, 512], F32, tag="s", name=f"qt{m}_{n}")
                for d in range(8):
                    nc.tensor.matmul(ps[:],
                                     lhsT=(wq[:, d, m * P:(m + 1) * P]),
                                     rhs=(xT[:, d, n * 512:(n + 1) * 512]),
                                     start=(d == 0), stop=(d == 7))
                nc.vector.tensor_copy(out=QT[:, m, n * 512:(n + 1) * 512],
                                      in_=ps[:])

            Vb = pp.tile([P, 16, DH], BF16, tag="Vb")

            def emit_v(t):
                ps = psS.tile([P, DH], F32, tag="s", name=f"v{t}")
                for d in range(8):
                    nc.tensor.matmul(ps[:],
                                     lhsT=(xT[:, d, t * P:(t + 1) * P]),
                                     rhs=(wv[:, d, :]),
                                     start=(d == 0), stop=(d == 7))
                nc.vector.tensor_copy(out=Vb[:, t, 0:DH], in_=ps[:])

            emit_kt(0)
            for n in (0, 1):
                emit_qt(0, n)
                for h in range(2):
                    emit_qt4(0, h, n)

            # ---- attention block state --------------------------------
            OTn = [op_.tile([P, NTOK], BF16, tag="otn", name=f"otn{m}")
                   for m in range(2)]

            on2_tiles = {}  # (qc, m, qch) -> On2 tile

            def emit_norm(od, den_sb, qc, m, h, split=False):
                rcp = wp.tile([P, 8], F32, tag="rcp", bufs=4)
                nc.vector.reciprocal(rcp[:], den_sb[:])
                for qch in range(8):
                    key = (qc, m, qch)
                    if key not in on2_tiles:
                        on2_tiles[key] = wp.tile(
                            [P, 2, DH], BF16, tag="on2", bufs=24,
                            name=f"on2_{qc}_{m}_{qch}")
                    dst = on2_tiles[key][:, h, :]
                    srcp = od[:, qch * DH:(qch + 1) * DH]
                    if split and qch % 2 == 1:
                        nc.scalar.activation(
                            dst, srcp, mybir.ActivationFunctionType.Copy,
                            scale=rcp[:, qch:qch + 1])
                    else:
                        nc.vector.tensor_scalar_mul(dst, srcp,
                                                    rcp[:, qch:qch + 1])

            def emit_transpose(qc, m, qch, copy_eng="dve"):
                on2 = on2_tiles.pop((qc, m, qch))
                ot = psS.tile([P, P], BF16, tag="s", name=f"ot{qc}{m}{qch}")
                nc.tensor.matmul(ot[:], lhsT=on2[:, :, :], rhs=identT[:],
                                 is_transpose=True)
                dst = OTn[m][:, qc * TQ + qch * P: qc * TQ + (qch + 1) * P]
                if copy_eng == "act":
                    nc.scalar.activation(dst, ot[:],
                                         mybir.ActivationFunctionType.Copy)
                else:
                    nc.vector.tensor_copy(out=dst, in_=ot[:])

            of_tiles = {}

            def emit_proj_half(t, n2, copy_eng="dve"):
                if t not in of_tiles:
                    of_tiles[t] = wp.tile([P, DIM], BF16, tag="of", bufs=10,
                                          name=f"of{t}")
                of = of_tiles[t]
                pf = psS.tile([P, 512], F32, tag="s", name=f"pf{t}_{n2}")
                for mm in range(2):
                    nc.tensor.matmul(
                        pf[:],
                        lhsT=(OTn[mm][:, t * P:(t + 1) * P]),
                        rhs=(wo2[:, mm, n2 * 512:(n2 + 1) * 512]),
                        start=(mm == 0), stop=(mm == 1))
                dst = of[:, n2 * 512:(n2 + 1) * 512]
                if copy_eng == "act":
                    nc.scalar.activation(dst, pf[:],
                                         mybir.ActivationFunctionType.Copy)
                else:
                    nc.vector.tensor_copy(out=dst, in_=pf[:])
                if n2 == 1:
                    of_tiles.pop(t)
                    nc.sync.dma_start(out_d[t * P:(t + 1) * P, :], of[:])

            def emit_proj(t, copy_eng="dve"):
                for n2 in range(2):
                    emit_proj_half(t, n2, copy_eng)

            # interleave hooks: (block_idx, kc) -> thunks emitted at the top
            # of that key iteration.  block_idx = qc*4 + m*2 + h.
            hooks = {}

            def add_hook(bi, kc, fn):
                hooks.setdefault((bi, kc), []).append(fn)

            # KT blocks just-in-time inside block 0; QT blocks spread over
            # the whole schedule at their first-use points so no single
            # act-paced block carries more than ~2us of projection work
            add_hook(0, 2, lambda: emit_kt(1))
            add_hook(0, 6, lambda: emit_kt(2))
            add_hook(0, 10, lambda: emit_kt(3))

            def qt_and_dup(m, n):
                emit_qt(m, n)
                for h in range(2):
                    emit_qt4(m, h, n)

            add_hook(1, 2, lambda: qt_and_dup(1, 0))
            add_hook(1, 8, lambda: qt_and_dup(1, 1))
            add_hook(2, 2, lambda: qt_and_dup(0, 2))
            add_hook(3, 2, lambda: qt_and_dup(0, 3))
            add_hook(4, 2, lambda: qt_and_dup(1, 2))
            add_hook(5, 2, lambda: qt_and_dup(1, 3))
            if KSTAGE >= 5:
                # transposes for pair (qc, m) run two blocks later
                for qch in range(8):
                    add_hook(2, 8 + (qch * 7) // 8,
                             lambda qch=qch: emit_transpose(0, 0, qch))
                    add_hook(4, 8 + (qch * 7) // 8,
                             lambda qch=qch: emit_transpose(0, 1, qch))
                    add_hook(6, 8 + (qch * 7) // 8,
                             lambda qch=qch: emit_transpose(1, 0, qch))
                # qc0 output rows while qc1 attention runs (half-proj per
                # slot to limit psum-ring contention)
                for i in range(8):
                    add_hook(5, 2 * i + 1,
                             lambda t=i: emit_proj_half(t, 0))
                    add_hook(6, 2 * i + 1,
                             lambda t=i: emit_proj_half(t, 1))

            kps = [None] * 16  # SBUF keep tiles of the current qc
            pending = []       # deferred (od, qc, m, h) normalizations
            gpipe = []         # cross-block PV pipeline (thunks)

            if KSTAGE < 2:
                # projections only; pad the rest with a dummy output
                for n in (1, 2, 3):
                    emit_kt(n)
                for m in range(2):
                    for n in range(4):
                        if (m, n) not in ((0, 0), (0, 1)):
                            emit_qt(m, n)
                for t in range(16):
                    emit_v(t)
                dummy = wp.tile([P, DIM], BF16, tag="of", bufs=3, name="dummy")
                nc.vector.memset(dummy[:], 0.0)
                for t in range(16):
                    nc.gpsimd.dma_start(out_d[t * P:(t + 1) * P, :], dummy[:])
                hooks.clear()
            for qc in range(2 if KSTAGE >= 2 else 0):
                for m in range(2):
                    for h in range(2):
                        bi = qc * 4 + m * 2 + h
                        od = psO.tile([P, 512], F32, tag="od",
                                      name=f"od{bi}")
                        den_sb = wp.tile([P, 8], F32, tag="den", bufs=4,
                                         name=f"den{bi}")
                        dpair = [None]

                        def emit_pv(kc, ee, od=od, den_sb=den_sb,
                                    dpair=dpair, bi=bi):
                            if kc % 2 == 0:
                                dpair[0] = psS.tile([P, 8], F32, tag="s",
                                                    name=f"dp{bi}_{kc}")
                            dp = dpair[0]
                            for qch in range(8):
                                first = (kc == 0 and qch == 0)
                                last = (kc == 15 and qch == 7)
                                lhsT = ee[:, qch * P:(qch + 1) * P]
                                nc.tensor.matmul(
                                    od[:, qch * DH:(qch + 1) * DH],
                                    lhsT=lhsT, rhs=Vb[:, kc, :],
                                    start=first, stop=last,
                                    skip_group_check=True)
                                nc.tensor.matmul(
                                    dp[:, qch:qch + 1],
                                    lhsT=lhsT, rhs=ones[:],
                                    start=(kc % 2 == 0 and qch == 0),
                                    stop=(kc % 2 == 1 and qch == 7),
                                    skip_group_check=True)
                            if kc % 2 == 1:
                                if kc == 1:
                                    nc.vector.tensor_copy(out=den_sb[:],
                                                          in_=dp[:])
                                else:
                                    nc.vector.tensor_add(den_sb[:],
                                                         den_sb[:], dp[:])

                        def fetch_kp(kc, qc=qc):
                            if KMASK == "dr":
                                kp = wp.tile([64, 2, TQ], F8, tag="kp",
                                             bufs=18, name=f"kp{qc}_{kc}")
                                nc.sync.dma_start(
                                    kp[:],
                                    km_d[kc * 64:(kc + 1) * 64, :,
                                         qc * TQ:(qc + 1) * TQ])
                            else:
                                kp = wp.tile([P, TQ], F8, tag="kp",
                                             bufs=18, name=f"kp{qc}_{kc}")
                                nc.sync.dma_start(
                                    kp[:],
                                    km_d.ap().rearrange("p i q -> (p i) q")
                                    [kc * P:(kc + 1) * P,
                                     qc * TQ:(qc + 1) * TQ])
                            kps[kc] = kp

                        if m == 0 and h == 0:
                            fetch_kp(0)
                            fetch_kp(1)
                        for kc in range(16):
                            if bi == 0 and h == 0:
                                emit_v(kc)
                            if kc == 7 and pending:
                                if KSTAGE >= 4:
                                    for args in pending:
                                        emit_norm(*args)
                                pending.clear()
                            for fn in hooks.get((bi, kc), ()):
                                fn()
                            if m == 0 and h == 0 and kc + 2 < 16:
                                fetch_kp(kc + 2)
                            ss = psS.tile([P, TQ], F32, tag="s")
                            nomask = (KMASK == "none")
                            for qh in range(2):
                                if KSCORE == "dr":
                                    nc.tensor.matmul(
                                        ss[:, qh * 512:(qh + 1) * 512],
                                        lhsT=(KT8[:, :, kc * P:(kc + 1) * P]),
                                        rhs=(QT8[:, 2 * m + h, :,
                                                 qc * TQ + qh * 512:
                                                 qc * TQ + (qh + 1) * 512]),
                                        start=True, stop=nomask,
                                        perf_mode=DRMODE)
                                else:
                                    nc.tensor.matmul(
                                        ss[:, qh * 512:(qh + 1) * 512],
                                        lhsT=(KT[:, kc * P:(kc + 1) * P]),
                                        rhs=(QT4[:, 2 * m + h,
                                                 qc * TQ + qh * 512:
                                                 qc * TQ + (qh + 1) * 512]),
                                        start=True, stop=nomask)
                            for qh in range(2 if not nomask else 0):
                                if KMASK == "dr":
                                    nc.tensor.matmul(
                                        ss[:, qh * 512:(qh + 1) * 512],
                                        lhsT=ident2[:],
                                        rhs=kps[kc][:, :, qh * 512:(qh + 1) * 512],
                                        start=False, stop=True,
                                        perf_mode=DRMODE)
                                else:
                                    nc.tensor.matmul(
                                        ss[:, qh * 512:(qh + 1) * 512],
                                        lhsT=identM[:],
                                        rhs=kps[kc][:, qh * 512:(qh + 1) * 512],
                                        start=False, stop=True)
                            ee = wp.tile([P, TQ], BF16, tag="ee", bufs=8)
                            # scores are doubled (K and Q both duplicated
                            # across the 128 contraction rows)
                            nc.scalar.activation(ee[:], ss[:], EXP,
                                                 scale=SCALE / 2)
                            gpipe.append(
                                lambda kc=kc, ee=ee, f=emit_pv: f(kc, ee))
                            depth = 5 if not (bi == 7 and kc >= 10) else 2
                            if KSTAGE >= 3 and len(gpipe) > depth:
                                gpipe.pop(0)()
                                if bi == 7 and kc >= 10 and gpipe and \
                                        len(gpipe) > depth:
                                    gpipe.pop(0)()
                        pending.append((od, den_sb, qc, m, h))

            if KSTAGE >= 3:
                for f in gpipe:
                    f()
            gpipe.clear()
            # tail: final normalization, last pair's transposes, qc1 rows,
            # DVE/Act split so neither serializes the drain
            if KSTAGE >= 4:
                for args in pending:
                    emit_norm(*args, split=True)
            pending.clear()
            if KSTAGE >= 5:
                for qch in range(8):
                    emit_transpose(1, 1, qch,
                                   copy_eng="act" if qch % 2 else "dve")
                    emit_proj(8 + qch,
                              copy_eng="act" if qch % 2 == 0 else "dve")
            elif KSTAGE >= 2:
                dummy = wp.tile([P, DIM], BF16, tag="of", bufs=3, name="dummy")
                nc.vector.memset(dummy[:], 0.0)
                for t in range(16):
                    nc.gpsimd.dma_start(out_d[t * P:(t + 1) * P, :], dummy[:])

    nc.compile()
    return nc


_NC_CACHE = None
_LAST_PARTS = None


def _assemble(parts, bo):
    out = np.stack([parts[0] + parts[1] + parts[2] + parts[3],
                    parts[4] + parts[5] + parts[6] + parts[7]])
    return (out + bo[None, None, :]).astype(np.float32)


def _get_nc():
    global _NC_CACHE
    if _NC_CACHE is None:
        _NC_CACHE = _build_kernel()
    return _NC_CACHE


_IDENT2 = None
_IDENTM = np.eye(P, dtype=np.float32).astype(ml_dtypes.float8_e4m3)


def _host_consts():
    global _IDENT2
    if _IDENT2 is None:
        i2 = np.zeros((64, 2, P), dtype=np.float32)
        for p in range(64):
            for i in range(2):
                i2[p, i, 2 * p + i] = 1.0
        _IDENT2 = i2.astype(ml_dtypes.float8_e4m3)
    identT = np.eye(P, dtype=np.float32).astype(ml_dtypes.bfloat16)
    return _IDENT2, identT


def kernel(x, mask, Wq, Wk, Wv, Wo, bo, _run_kwargs=None):
    x = np.asarray(x, dtype=np.float32)
    mask = np.asarray(mask).astype(bool)
    Wq = np.asarray(Wq, dtype=np.float32)
    Wk = np.asarray(Wk, dtype=np.float32)
    Wv = np.asarray(Wv, dtype=np.float32)
    Wo = np.asarray(Wo, dtype=np.float32)
    bo = np.asarray(bo, dtype=np.float32)

    nc = _get_nc()

    ident2, identT = _host_consts()
    # ss tile is S^T [key, query]; reference masks where mask[query, key].
    keepM2 = np.ascontiguousarray(
        -240.0 * mask.T.astype(np.float32)).astype(
        ml_dtypes.float8_e4m3).reshape(NTOK // 2, 2, NTOK)

    in_maps = []
    for c in range(N_CORES):
        b, j = c // 4, c % 4
        in_maps.append({
            "xT": np.ascontiguousarray(x[b].T).astype(ml_dtypes.bfloat16),
            "keepM2": keepM2,
            "ident2": ident2,
            "identT": identT,
            "identM": _IDENTM,
            "wq": np.ascontiguousarray(
                Wq[:, j * 256:(j + 1) * 256]).astype(ml_dtypes.bfloat16),
            "wk2": np.ascontiguousarray(
                np.concatenate([Wk[:, j * DH:(j + 1) * DH]] * 2,
                               axis=1)).astype(ml_dtypes.bfloat16),
            "wv": np.ascontiguousarray(
                Wv[:, j * DH:(j + 1) * DH]).astype(ml_dtypes.bfloat16),
            "wo": np.ascontiguousarray(
                Wo[j * 256:(j + 1) * 256, :]).astype(ml_dtypes.bfloat16),
        })

    res = run_bass_kernel_spmd(nc, in_maps, list(range(N_CORES)),
                               **(_run_kwargs or {}))
    parts = [res.results[c]["out"].astype(np.float32) for c in range(N_CORES)]
    global _LAST_PARTS
    _LAST_PARTS = parts
    out = _assemble(parts, bo)
    if _run_kwargs:
        kernel.last_results = res
    return out


if __name__ == "__main__":
    pass


# revision 27
# speedup vs baseline: 1.0029x; 1.0029x over previous
"""GQA attention kernel for 8 TRN2 NeuronCores.

Problem: B=2, N=2048, DIM=1024, 16 q-heads / 4 kv-heads, head dim 64.
Sharding: core c handles batch c//4 and kv-head group c%4 (4 q-heads sharing
one kv head).  Wq/Wk/Wv column-sharded, Wo row-sharded; the Wo row reduction
(4 cores per batch) and the bias add happen on the host.

Per-core algorithm:
  KT = Wk_dup.T @ x.T            [128, 2048]   (kv head duplicated twice)
  QT = Wq_shard.T @ x.T          [256, 2048]   (4 heads stacked as 2x128)
  V  = x @ Wv_shard              [2048, 64]  (bf16, key chunks on partitions)
  per head h, 1024-wide q block qc:
    for each 128-key chunk kc:
      ss[k, q]  = K.Q^T           (bf16 matmuls, psum [128, 1024])
                + mask            (fp8 DoubleRow matmul: -240 on masked
                                   entries; exp(scale*(s-240)) ~ 0)
      ee = exp(ss/8)  (Act engine, bf16)
      O[q, d]  += ee_chunk.T @ V_chunk   (8 matmuls out [128, 64], cheap
                                          free dim = 64 instead of 512)
      den[q]   += ee_chunk.T @ ones      (1-row matmuls into D columns)
    O and den live in one psum slot: bank X = O [128, 8, 64],
    bank X+1 = D [128, 8].
    normalize: rcp = 1/D (DVE), On2[:, h, :] = O * rcp (per-partition scalar)
  per (qc, m=head pair, qch): PE-transpose On2 [128(q), 2x64] -> OT
    [128(h,d), 128(q)] (bf16 psum), DVE copy into OTn[m].
  out rows t: pf = OTn[0].T_t @ Wo_pair0 + OTn[1].T_t @ Wo_pair1 (psum),
    DVE copy -> bf16, DMA out.

Emission is software-pipelined: per kc we emit scores(kc), exp(kc), then
PV(kc-2), so the PE never waits on the Activation engine.  Block
post-processing (normalize / transpose / projection) is interleaved into
subsequent blocks' key loops.
"""

import os
import sys

for _p in ("/opt/trn_rl_repo",):
    if _p not in sys.path:
        sys.path.insert(0, _p)

import numpy as np
import ml_dtypes

import concourse.bass as bass  # noqa: F401  (registers AP machinery)
import concourse.tile as tile
from concourse import bacc, mybir
from concourse.bass_utils import run_bass_kernel_spmd

F32 = mybir.dt.float32
F8 = mybir.dt.float8e4
BF16 = mybir.dt.bfloat16
EXP = mybir.ActivationFunctionType.Exp
DRMODE = mybir.MatmulPerfMode.DoubleRow

B, NTOK, DIM = 2, 2048, 1024
H, KVH, DH = 16, 4, 64
P = 128
TQ = 1024  # q-block width for the attention inner loop
SCALE = DH ** -0.5

N_CORES = 8
# bisect stages: 1=projections only, 2=+scores/mask/exp, 3=+PV/den,
# 4=+norm, 5=full (default)
KSTAGE = int(os.environ.get("KSTAGE", "5"))
# mask variants: dr (DoubleRow fp8), f8 (standard fp8 matmul), none
KMASK = os.environ.get("KMASK", "dr")
# score variants: dr (DoubleRow fp8 K/Q), bf16 (via QT4)
KSCORE = os.environ.get("KSCORE", "bf16")


def _build_kernel():
    nc = bacc.Bacc("TRN2", target_bir_lowering=False, debug=False,
                   num_devices=N_CORES)

    xT_d = nc.dram_tensor("xT", [DIM, NTOK], BF16, kind="ExternalInput")
    km_d = nc.dram_tensor("keepM2", [NTOK // 2, 2, NTOK], F8,
                          kind="ExternalInput")
    i2_d = nc.dram_tensor("ident2", [64, 2, P], F8, kind="ExternalInput")
    im_d = nc.dram_tensor("identM", [P, P], F8, kind="ExternalInput")
    it_d = nc.dram_tensor("identT", [P, P], BF16, kind="ExternalInput")
    wq_d = nc.dram_tensor("wq", [DIM, 256], BF16, kind="ExternalInput")
    wk_d = nc.dram_tensor("wk2", [DIM, 128], BF16, kind="ExternalInput")
    wv_d = nc.dram_tensor("wv", [DIM, DH], BF16, kind="ExternalInput")
    wo_d = nc.dram_tensor("wo", [256, DIM], BF16, kind="ExternalInput")
    out_d = nc.dram_tensor("out", [NTOK, DIM], BF16, kind="ExternalOutput")

    with tile.TileContext(nc) as tc:
        with tc.tile_pool(name="persist", bufs=1) as pp, \
             tc.tile_pool(name="work", bufs=3) as wp, \
             tc.tile_pool(name="otnp", bufs=2) as op_, \
             tc.tile_pool(name="psS", bufs=3, space="PSUM") as psS, \
             tc.tile_pool(name="psO", bufs=2, space="PSUM") as psO:
            # ---- resident tensors -------------------------------------
            # head is DMA-bound: spread the first xT blocks over all four
            # DMA-capable queues, weights needed first go first.
            wk = pp.tile([P, 8, P], BF16, tag="wk")
            nc.scalar.dma_start(wk[:], wk_d.ap().rearrange("(o p) m -> p o m", p=P))
            xT = pp.tile([P, 8, NTOK], BF16, tag="xT")
            # 512KB grouped transfers: DMA dispatch is ~625ns/transfer, so
            # fewer, bigger chunks land the first q-block much sooner.
            xT_view = xT_d.ap().rearrange("(o p) m -> p o m", p=P)
            for nq in range(2):
                for og in range(2):
                    eng = nc.sync if og == 0 else nc.gpsimd
                    eng.dma_start(
                        xT[:, og * 4:(og + 1) * 4, nq * 512:(nq + 1) * 512],
                        xT_view[:, og * 4:(og + 1) * 4,
                                nq * 512:(nq + 1) * 512])
            wq = pp.tile([P, 8, 256], BF16, tag="wq")
            nc.sync.dma_start(wq[:], wq_d.ap().rearrange("(o p) m -> p o m", p=P))
            ident2 = pp.tile([64, 2, P], F8, tag="ident2")
            nc.scalar.dma_start(ident2[:], i2_d[:, :, :])
            identM = pp.tile([P, P], F8, tag="identM")
            nc.scalar.dma_start(identM[:], im_d[:, :])
            for nq in range(2, 4):
                for og in range(2):
                    eng = nc.sync if og == 0 else nc.gpsimd
                    eng.dma_start(
                        xT[:, og * 4:(og + 1) * 4, nq * 512:(nq + 1) * 512],
                        xT_view[:, og * 4:(og + 1) * 4,
                                nq * 512:(nq + 1) * 512])
            identT = pp.tile([P, P], BF16, tag="identT")
            nc.scalar.dma_start(identT[:], it_d[:, :])
            wv = pp.tile([P, 8, DH], BF16, tag="wv")
            nc.scalar.dma_start(wv[:], wv_d.ap().rearrange("(o p) m -> p o m", p=P))
            wo2 = pp.tile([P, 2, DIM], BF16, tag="wo2")
            for mm in range(2):
                nc.scalar.dma_start(wo2[:, mm, :],
                                    wo_d[mm * P:(mm + 1) * P, :])
            ones = pp.tile([P, 1], BF16, tag="ones")
            nc.vector.memset(ones[:], 1.0)

            # ---- projections ------------------------------------------
            KT = pp.tile([P, NTOK], BF16, tag="KT")

            def emit_kt(n):
                ps = psS.tile([P, 512], F32, tag="s", name=f"kt{n}")
                for d in range(8):
                    nc.tensor.matmul(ps[:], lhsT=(wk[:, d, :]),
                                     rhs=(xT[:, d, n * 512:(n + 1) * 512]),
                                     start=(d == 0), stop=(d == 7))
                nc.vector.tensor_copy(out=KT[:, n * 512:(n + 1) * 512], in_=ps[:])
                if KSCORE == "dr":
                    emit_kt8(n)

            QT = pp.tile([P, 2, NTOK], BF16, tag="QT")
            # duplicated-head layout: head hh twice along partitions, so score
            # matmuls use full 128-row tiles (DR matmuls break on HW when
            # mixed with 64-row tiles at partition 64 + psum slot reuse)
            QT4 = pp.tile([P, 4, NTOK], BF16, tag="QT4")
            # fp8 DoubleRow layouts: contraction (p, i) with the head dim
            # replicated in the i slot (scores come out doubled, same as the
            # bf16 dup path; exp scale folds the factor away).  fp8
            # conversions run on the otherwise-idle Pool engine (SBUF only).
            KT8 = pp.tile([64, 2, NTOK], F8, tag="KT8")
            QT8 = pp.tile([64, 4, 2, NTOK], F8, tag="QT8")
            QTH = pp.tile([64, 2, NTOK], BF16, tag="QTH")

            def emit_kt8(n):
                for i in range(2):
                    nc.gpsimd.tensor_copy(
                        out=KT8[:, i, n * 512:(n + 1) * 512],
                        in_=KT[0:64, n * 512:(n + 1) * 512])

            def emit_qt8(m, h, n):
                hh = 2 * m + h
                sl = slice(n * 512, (n + 1) * 512)
                if h == 0:
                    srcq = QT[0:64, m, sl]
                else:
                    nc.sync.dma_start(QTH[:, m, sl], QT[64:128, m, sl])
                    srcq = QTH[:, m, sl]
                for i in range(2):
                    nc.gpsimd.tensor_copy(out=QT8[:, hh, i, sl], in_=srcq)

            def emit_qt4(m, h, n):
                if KSCORE == "dr":
                    emit_qt8(m, h, n)
                    return
                hh = 2 * m + h
                src_ap = QT[h * DH:(h + 1) * DH, m, n * 512:(n + 1) * 512]
                eng = nc.gpsimd if (n + h) % 2 else nc.sync
                eng.dma_start(QT4[0:64, hh, n * 512:(n + 1) * 512], src_ap)
                eng.dma_start(QT4[64:128, hh, n * 512:(n + 1) * 512], src_ap)

            def emit_qt(m, n):
                ps = psS.tile([P, 512], F32, tag="s", name=f"qt{m}_{n}")
                for d in range(8):
                    nc.tensor.matmul(ps[:],
                                     lhsT=(wq[:, d, m * P:(m + 1) * P]),
                                     rhs=(xT[:, d, n * 512:(n + 1) * 512]),
                                     start=(d == 0), stop=(d == 7))
                nc.vector.tensor_copy(out=QT[:, m, n * 512:(n + 1) * 512],
                                      in_=ps[:])

            Vb = pp.tile([P, 16, DH], BF16, tag="Vb")

            def emit_v(t):
                ps = psS.tile([P, DH], F32, tag="s", name=f"v{t}")
                for d in range(8):
                    nc.tensor.matmul(ps[:],
                                     lhsT=(xT[:, d, t * P:(t + 1) * P]),
                                     rhs=(wv[:, d, :]),
                                     start=(d == 0), stop=(d == 7))
                nc.vector.tensor_copy(out=Vb[:, t, 0:DH], in_=ps[:])

            emit_kt(0)
            for n in (0, 1):
                emit_qt(0, n)
                for h in range(2):
                    emit_qt4(0, h, n)

            # ---- attention block state --------------------------------
            OTn = [op_.tile([P, NTOK], BF16, tag="otn", name=f"otn{m}")
                   for m in range(2)]

            on2_tiles = {}  # (qc, m, qch) -> On2 tile

            def emit_norm(od, den_sb, qc, m, h, split=False):
                rcp = wp.tile([P, 8], F32, tag="rcp", bufs=4)
                nc.vector.reciprocal(rcp[:], den_sb[:])
                for qch in range(8):
                    key = (qc, m, qch)
                    if key not in on2_tiles:
                        on2_tiles[key] = wp.tile(
                            [P, 2, DH], BF16, tag="on2", bufs=24,
                            name=f"on2_{qc}_{m}_{qch}")
                    dst = on2_tiles[key][:, h, :]
                    srcp = od[:, qch * DH:(qch + 1) * DH]
                    if split and qch % 2 == 1:
                        nc.scalar.activation(
                            dst, srcp, mybir.ActivationFunctionType.Copy,
                            scale=rcp[:, qch:qch + 1])
                    else:
                        nc.vector.tensor_scalar_mul(dst, srcp,
                                                    rcp[:, qch:qch + 1])

            def emit_transpose(qc, m, qch, copy_eng="dve"):
                on2 = on2_tiles.pop((qc, m, qch))
                ot = psS.tile([P, P], BF16, tag="s", name=f"ot{qc}{m}{qch}")
                nc.tensor.matmul(ot[:], lhsT=on2[:, :, :], rhs=identT[:],
                                 is_transpose=True)
                dst = OTn[m][:, qc * TQ + qch * P: qc * TQ + (qch + 1) * P]
                if copy_eng == "act":
                    nc.scalar.activation(dst, ot[:],
                                         mybir.ActivationFunctionType.Copy)
                else:
                    nc.vector.tensor_copy(out=dst, in_=ot[:])

            of_tiles = {}

            def emit_proj_half(t, n2, copy_eng="dve"):
                if t not in of_tiles:
                    of_tiles[t] = wp.tile([P, DIM], BF16, tag="of", bufs=10,
                                          name=f"of{t}")
                of = of_tiles[t]
                pf = psS.tile([P, 512], F32, tag="s", name=f"pf{t}_{n2}")
                for mm in range(2):
                    nc.tensor.matmul(
                        pf[:],
                        lhsT=(OTn[mm][:, t * P:(t + 1) * P]),
                        rhs=(wo2[:, mm, n2 * 512:(n2 + 1) * 512]),
                        start=(mm == 0), stop=(mm == 1))
                dst = of[:, n2 * 512:(n2 + 1) * 512]
                if copy_eng == "act":
                    nc.scalar.activation(dst, pf[:],
                                         mybir.ActivationFunctionType.Copy)
                else:
                    nc.vector.tensor_copy(out=dst, in_=pf[:])
                if n2 == 1:
                    of_tiles.pop(t)
                    nc.sync.dma_start(out_d[t * P:(t + 1) * P, :], of[:])

            def emit_proj(t, copy_eng="dve"):
                for n2 in range(2):
                    emit_proj_half(t, n2, copy_eng)

            # interleave hooks: (block_idx, kc) -> thunks emitted at the top
            # of that key iteration.  block_idx = qc*4 + m*2 + h.
            hooks = {}

            def add_hook(bi, kc, fn):
                hooks.setdefault((bi, kc), []).append(fn)

            # KT blocks just-in-time inside block 0; QT blocks spread over
            # the whole schedule at their first-use points so no single
            # act-paced block carries more than ~2us of projection work
            add_hook(0, 2, lambda: emit_kt(1))
            add_hook(0, 6, lambda: emit_kt(2))
            add_hook(0, 10, lambda: emit_kt(3))

            def qt_and_dup(m, n):
                emit_qt(m, n)
                for h in range(2):
                    emit_qt4(m, h, n)

            add_hook(1, 2, lambda: qt_and_dup(1, 0))
            add_hook(1, 8, lambda: qt_and_dup(1, 1))
            add_hook(2, 2, lambda: qt_and_dup(0, 2))
            add_hook(3, 2, lambda: qt_and_dup(0, 3))
            add_hook(4, 2, lambda: qt_and_dup(1, 2))
            add_hook(5, 2, lambda: qt_and_dup(1, 3))
            if KSTAGE >= 5:
                # transposes for pair (qc, m) run two blocks later
                for qch in range(8):
                    add_hook(2, 8 + (qch * 7) // 8,
                             lambda qch=qch: emit_transpose(0, 0, qch))
                    add_hook(4, 8 + (qch * 7) // 8,
                             lambda qch=qch: emit_transpose(0, 1, qch))
                    add_hook(6, 8 + (qch * 7) // 8,
                             lambda qch=qch: emit_transpose(1, 0, qch))
                # qc0 output rows while qc1 attention runs (half-proj per
                # slot to limit psum-ring contention)
                for i in range(8):
                    add_hook(5, 2 * i + 1,
                             lambda t=i: emit_proj_half(t, 0))
                    add_hook(6, 2 * i + 1,
                             lambda t=i: emit_proj_half(t, 1))

            kps = [None] * 16  # SBUF keep tiles of the current qc
            pending = []       # deferred (od, qc, m, h) normalizations
            gpipe = []         # cross-block PV pipeline (thunks)

            if KSTAGE < 2:
                # projections only; pad the rest with a dummy output
                for n in (1, 2, 3):
                    emit_kt(n)
                for m in range(2):
                    for n in range(4):
                        if (m, n) not in ((0, 0), (0, 1)):
                            emit_qt(m, n)
                for t in range(16):
                    emit_v(t)
                dummy = wp.tile([P, DIM], BF16, tag="of", bufs=3, name="dummy")
                nc.vector.memset(dummy[:], 0.0)
                for t in range(16):
                    nc.gpsimd.dma_start(out_d[t * P:(t + 1) * P, :], dummy[:])
                hooks.clear()
            for qc in range(2 if KSTAGE >= 2 else 0):
                for m in range(2):
                    for h in range(2):
                        bi = qc * 4 + m * 2 + h
                        od = psO.tile([P, 512], F32, tag="od",
                                      name=f"od{bi}")
                        den_sb = wp.tile([P, 8], F32, tag="den", bufs=4,
                                         name=f"den{bi}")
                        dpair = [None]

                        def emit_pv(kc, ee, od=od, den_sb=den_sb,
                                    dpair=dpair, bi=bi):
                            if kc % 2 == 0:
                                dpair[0] = psS.tile([P, 8], F32, tag="s",
                                                    name=f"dp{bi}_{kc}")
                            dp = dpair[0]
                            for qch in range(8):
                                first = (kc == 0 and qch == 0)
                                last = (kc == 15 and qch == 7)
                                lhsT = ee[:, qch * P:(qch + 1) * P]
                                nc.tensor.matmul(
                                    od[:, qch * DH:(qch + 1) * DH],
                                    lhsT=lhsT, rhs=Vb[:, kc, :],
                                    start=first, stop=last,
                                    skip_group_check=True)
                                nc.tensor.matmul(
                                    dp[:, qch:qch + 1],
                                    lhsT=lhsT, rhs=ones[:],
                                    start=(kc % 2 == 0 and qch == 0),
                                    stop=(kc % 2 == 1 and qch == 7),
                                    skip_group_check=True)
                            if kc % 2 == 1:
                                if kc == 1:
                                    nc.vector.tensor_copy(out=den_sb[:],
                                                          in_=dp[:])
                                else:
                                    nc.vector.tensor_add(den_sb[:],
                                                         den_sb[:], dp[:])

                        def fetch_kp(kc, qc=qc):
                            if KMASK == "dr":
                                kp = wp.tile([64, 2, TQ], F8, tag="kp",
                                             bufs=18, name=f"kp{qc}_{kc}")
                                nc.sync.dma_start(
                                    kp[:],
                                    km_d[kc * 64:(kc + 1) * 64, :,
                                         qc * TQ:(qc + 1) * TQ])
                            else:
                                kp = wp.tile([P, TQ], F8, tag="kp",
                                             bufs=18, name=f"kp{qc}_{kc}")
                                nc.sync.dma_start(
                                    kp[:],
                                    km_d.ap().rearrange("p i q -> (p i) q")
                                    [kc * P:(kc + 1) * P,
                                     qc * TQ:(qc + 1) * TQ])
                            kps[kc] = kp

                        if m == 0 and h == 0:
                            fetch_kp(0)
                            fetch_kp(1)
                        for kc in range(16):
                            if bi == 0 and h == 0:
                                emit_v(kc)
                            if kc == 7 and pending:
                                if KSTAGE >= 4:
                                    for args in pending:
                                        emit_norm(*args)
                                pending.clear()
                            for fn in hooks.get((bi, kc), ()):
                                fn()
                            if m == 0 and h == 0 and kc + 2 < 16:
                                fetch_kp(kc + 2)
                            ss = psS.tile([P, TQ], F32, tag="s")
                            nomask = (KMASK == "none")
                            for qh in range(2):
                                if KSCORE == "dr":
                                    nc.tensor.matmul(
                                        ss[:, qh * 512:(qh + 1) * 512],
                                        lhsT=(KT8[:, :, kc * P:(kc + 1) * P]),
                                        rhs=(QT8[:, 2 * m + h, :,
                                                 qc * TQ + qh * 512:
                                                 qc * TQ + (qh + 1) * 512]),
                                        start=True, stop=nomask,
                                        perf_mode=DRMODE)
                                else:
                                    nc.tensor.matmul(
                                        ss[:, qh * 512:(qh + 1) * 512],
                                        lhsT=(KT[:, kc * P:(kc + 1) * P]),
                                        rhs=(QT4[:, 2 * m + h,
                                                 qc * TQ + qh * 512:
                                                 qc * TQ + (qh + 1) * 512]),
                                        start=True, stop=nomask)
                            for qh in range(2 if not nomask else 0):
                                if KMASK == "dr":
                                    nc.tensor.matmul(
                                        ss[:, qh * 512:(qh + 1) * 512],
                                        lhsT=ident2[:],
                                        rhs=kps[kc][:, :, qh * 512:(qh + 1) * 512],
                                        start=False, stop=True,
                                        perf_mode=DRMODE)
                                else:
                                    nc.tensor.matmul(
                                        ss[:, qh * 512:(qh + 1) * 512],
                                        lhsT=identM[:],
                                        rhs=kps[kc][:, qh * 512:(qh + 1) * 512],
                                        start=False, stop=True)
                            ee = wp.tile([P, TQ], BF16, tag="ee", bufs=10)
                            # scores are doubled (K and Q both duplicated
                            # across the 128 contraction rows)
                            nc.scalar.activation(ee[:], ss[:], EXP,
                                                 scale=SCALE / 2)
                            gpipe.append(
                                lambda kc=kc, ee=ee, f=emit_pv: f(kc, ee))
                            depth = 7 if not (bi == 7 and kc >= 8) else 2
                            if KSTAGE >= 3 and len(gpipe) > depth:
                                gpipe.pop(0)()
                                if bi == 7 and kc >= 10 and gpipe and \
                                        len(gpipe) > depth:
                                    gpipe.pop(0)()
                        pending.append((od, den_sb, qc, m, h))

            if KSTAGE >= 3:
                for f in gpipe:
                    f()
            gpipe.clear()
            # tail: final normalization, last pair's transposes, qc1 rows,
            # DVE/Act split so neither serializes the drain
            if KSTAGE >= 4:
                for args in pending:
                    emit_norm(*args, split=True)
            pending.clear()
            if KSTAGE >= 5:
                for qch in range(8):
                    emit_transpose(1, 1, qch,
                                   copy_eng="act" if qch % 2 else "dve")
                    emit_proj(8 + qch,
                              copy_eng="act" if qch % 2 == 0 else "dve")
            elif KSTAGE >= 2:
                dummy = wp.tile([P, DIM], BF16, tag="of", bufs=3, name="dummy")
                nc.vector.memset(dummy[:], 0.0)
                for t in range(16):
                    nc.gpsimd.dma_start(out_d[t * P:(t + 1) * P, :], dummy[:])

    nc.compile()
    return nc


_NC_CACHE = None
_LAST_PARTS = None


def _assemble(parts, bo):
    out = np.stack([parts[0] + parts[1] + parts[2] + parts[3],
                    parts[4] + parts[5] + parts[6] + parts[7]])
    return (out + bo[None, None, :]).astype(np.float32)


def _get_nc():
    global _NC_CACHE
    if _NC_CACHE is None:
        _NC_CACHE = _build_kernel()
    return _NC_CACHE


_IDENT2 = None
_IDENTM = np.eye(P, dtype=np.float32).astype(ml_dtypes.float8_e4m3)


def _host_consts():
    global _IDENT2
    if _IDENT2 is None:
        i2 = np.zeros((64, 2, P), dtype=np.float32)
        for p in range(64):
            for i in range(2):
                i2[p, i, 2 * p + i] = 1.0
        _IDENT2 = i2.astype(ml_dtypes.float8_e4m3)
    identT = np.eye(P, dtype=np.float32).astype(ml_dtypes.bfloat16)
    return _IDENT2, identT


def kernel(x, mask, Wq, Wk, Wv, Wo, bo, _run_kwargs=None):
    x = np.asarray(x, dtype=np.float32)
    mask = np.asarray(mask).astype(bool)
    Wq = np.asarray(Wq, dtype=np.float32)
    Wk = np.asarray(Wk, dtype=np.float32)
    Wv = np.asarray(Wv, dtype=np.float32)
    Wo = np.asarray(Wo, dtype=np.float32)
    bo = np.asarray(bo, dtype=np.float32)

    nc = _get_nc()

    ident2, identT = _host_consts()
    # ss tile is S^T [key, query]; reference masks where mask[query, key].
    keepM2 = np.ascontiguousarray(
        -240.0 * mask.T.astype(np.float32)).astype(
        ml_dtypes.float8_e4m3).reshape(NTOK // 2, 2, NTOK)

    in_maps = []
    for c in range(N_CORES):
        b, j = c // 4, c % 4
        in_maps.append({
            "xT": np.ascontiguousarray(x[b].T).astype(ml_dtypes.bfloat16),
            "keepM2": keepM2,
            "ident2": ident2,
            "identT": identT,
            "identM": _IDENTM,
            "wq": np.ascontiguousarray(
                Wq[:, j * 256:(j + 1) * 256]).astype(ml_dtypes.bfloat16),
            "wk2": np.ascontiguousarray(
                np.concatenate([Wk[:, j * DH:(j + 1) * DH]] * 2,
                               axis=1)).astype(ml_dtypes.bfloat16),
            "wv": np.ascontiguousarray(
                Wv[:, j * DH:(j + 1) * DH]).astype(ml_dtypes.bfloat16),
            "wo": np.ascontiguousarray(
                Wo[j * 256:(j + 1) * 256, :]).astype(ml_dtypes.bfloat16),
        })

    res = run_bass_kernel_spmd(nc, in_maps, list(range(N_CORES)),
                               **(_run_kwargs or {}))
    parts = [res.results[c]["out"].astype(np.float32) for c in range(N_CORES)]
    global _LAST_PARTS
    _LAST_PARTS = parts
    out = _assemble(parts, bo)
    if _run_kwargs:
        kernel.last_results = res
    return out


if __name__ == "__main__":
    pass


# revision 28
# speedup vs baseline: 1.0230x; 1.0200x over previous
"""GQA attention kernel for 8 TRN2 NeuronCores.

Problem: B=2, N=2048, DIM=1024, 16 q-heads / 4 kv-heads, head dim 64.
Sharding: core c handles batch c//4 and kv-head group c%4 (4 q-heads sharing
one kv head).  Wq/Wk/Wv column-sharded, Wo row-sharded; the Wo row reduction
(4 cores per batch) and the bias add happen on the host.

Per-core algorithm:
  KT = Wk_dup.T @ x.T            [128, 2048]   (kv head duplicated twice)
  QT = Wq_shard.T @ x.T          [256, 2048]   (4 heads stacked as 2x128)
  V  = x @ Wv_shard              [2048, 64]  (bf16, key chunks on partitions)
  per head h, 1024-wide q block qc:
    for each 128-key chunk kc:
      ss[k, q]  = K.Q^T           (bf16 matmuls, psum [128, 1024])
                + mask            (fp8 DoubleRow matmul: -240 on masked
                                   entries; exp(scale*(s-240)) ~ 0)
      ee = exp(ss/8)  (Act engine, bf16)
      O[q, d]  += ee_chunk.T @ V_chunk   (8 matmuls out [128, 64], cheap
                                          free dim = 64 instead of 512)
      den[q]   += ee_chunk.T @ ones      (1-row matmuls into D columns)
    O and den live in one psum slot: bank X = O [128, 8, 64],
    bank X+1 = D [128, 8].
    normalize: rcp = 1/D (DVE), On2[:, h, :] = O * rcp (per-partition scalar)
  per (qc, m=head pair, qch): PE-transpose On2 [128(q), 2x64] -> OT
    [128(h,d), 128(q)] (bf16 psum), DVE copy into OTn[m].
  out rows t: pf = OTn[0].T_t @ Wo_pair0 + OTn[1].T_t @ Wo_pair1 (psum),
    DVE copy -> bf16, DMA out.

Emission is software-pipelined: per kc we emit scores(kc), exp(kc), then
PV(kc-2), so the PE never waits on the Activation engine.  Block
post-processing (normalize / transpose / projection) is interleaved into
subsequent blocks' key loops.
"""

import os
import sys

for _p in ("/opt/trn_rl_repo",):
    if _p not in sys.path:
        sys.path.insert(0, _p)

import numpy as np
import ml_dtypes

import concourse.bass as bass  # noqa: F401  (registers AP machinery)
import concourse.tile as tile
from concourse import bacc, mybir
from concourse.bass_utils import run_bass_kernel_spmd

F32 = mybir.dt.float32
F8 = mybir.dt.float8e4
BF16 = mybir.dt.bfloat16
EXP = mybir.ActivationFunctionType.Exp
DRMODE = mybir.MatmulPerfMode.DoubleRow

B, NTOK, DIM = 2, 2048, 1024
H, KVH, DH = 16, 4, 64
P = 128
TQ = 1024  # q-block width for the attention inner loop
SCALE = DH ** -0.5

N_CORES = 8
# bisect stages: 1=projections only, 2=+scores/mask/exp, 3=+PV/den,
# 4=+norm, 5=full (default)
KSTAGE = int(os.environ.get("KSTAGE", "5"))
# mask variants: dr (DoubleRow fp8), f8 (standard fp8 matmul), none
KMASK = os.environ.get("KMASK", "dr")
# score variants: dr (DoubleRow fp8 K/Q), bf16 (via QT4)
KSCORE = os.environ.get("KSCORE", "bf16")


def _build_kernel():
    nc = bacc.Bacc("TRN2", target_bir_lowering=False, debug=False,
                   num_devices=N_CORES)

    xT_d = nc.dram_tensor("xT", [DIM, NTOK], BF16, kind="ExternalInput")
    km_d = nc.dram_tensor("keepM2", [NTOK // 2, 2, NTOK], F8,
                          kind="ExternalInput")
    i2_d = nc.dram_tensor("ident2", [64, 2, P], F8, kind="ExternalInput")
    im_d = nc.dram_tensor("identM", [P, P], F8, kind="ExternalInput")
    it_d = nc.dram_tensor("identT", [P, P], BF16, kind="ExternalInput")
    wq_d = nc.dram_tensor("wq", [DIM, 256], BF16, kind="ExternalInput")
    wk_d = nc.dram_tensor("wk2", [DIM, 128], BF16, kind="ExternalInput")
    wv_d = nc.dram_tensor("wv", [DIM, DH], BF16, kind="ExternalInput")
    wo_d = nc.dram_tensor("wo", [256, DIM], BF16, kind="ExternalInput")
    out_d = nc.dram_tensor("out", [NTOK, DIM], BF16, kind="ExternalOutput")

    with tile.TileContext(nc) as tc:
        with tc.tile_pool(name="persist", bufs=1) as pp, \
             tc.tile_pool(name="work", bufs=3) as wp, \
             tc.tile_pool(name="otnp", bufs=2) as op_, \
             tc.tile_pool(name="psS", bufs=3, space="PSUM") as psS, \
             tc.tile_pool(name="psO", bufs=2, space="PSUM") as psO:
            # ---- resident tensors -------------------------------------
            # head is DMA-bound: spread the first xT blocks over all four
            # DMA-capable queues, weights needed first go first.
            wk = pp.tile([P, 8, P], BF16, tag="wk")
            nc.scalar.dma_start(wk[:], wk_d.ap().rearrange("(o p) m -> p o m", p=P))
            xT = pp.tile([P, 8, NTOK], BF16, tag="xT")
            # 512KB grouped transfers: DMA dispatch is ~625ns/transfer, so
            # fewer, bigger chunks land the first q-block much sooner.
            xT_view = xT_d.ap().rearrange("(o p) m -> p o m", p=P)
            for nq in range(2):
                for og in range(2):
                    eng = nc.sync if og == 0 else nc.gpsimd
                    eng.dma_start(
                        xT[:, og * 4:(og + 1) * 4, nq * 512:(nq + 1) * 512],
                        xT_view[:, og * 4:(og + 1) * 4,
                                nq * 512:(nq + 1) * 512])
            wq = pp.tile([P, 8, 256], BF16, tag="wq")
            nc.sync.dma_start(wq[:], wq_d.ap().rearrange("(o p) m -> p o m", p=P))
            ident2 = pp.tile([64, 2, P], F8, tag="ident2")
            nc.scalar.dma_start(ident2[:], i2_d[:, :, :])
            identM = pp.tile([P, P], F8, tag="identM")
            nc.scalar.dma_start(identM[:], im_d[:, :])
            for nq in range(2, 4):
                for og in range(2):
                    eng = nc.sync if og == 0 else nc.gpsimd
                    eng.dma_start(
                        xT[:, og * 4:(og + 1) * 4, nq * 512:(nq + 1) * 512],
                        xT_view[:, og * 4:(og + 1) * 4,
                                nq * 512:(nq + 1) * 512])
            identT = pp.tile([P, P], BF16, tag="identT")
            nc.scalar.dma_start(identT[:], it_d[:, :])
            wv = pp.tile([P, 8, DH], BF16, tag="wv")
            nc.scalar.dma_start(wv[:], wv_d.ap().rearrange("(o p) m -> p o m", p=P))
            wo2 = pp.tile([P, 2, DIM], BF16, tag="wo2")
            for mm in range(2):
                nc.scalar.dma_start(wo2[:, mm, :],
                                    wo_d[mm * P:(mm + 1) * P, :])
            ones = pp.tile([P, 1], BF16, tag="ones")
            nc.vector.memset(ones[:], 1.0)

            # ---- projections ------------------------------------------
            KT = pp.tile([P, NTOK], BF16, tag="KT")

            def emit_kt(n):
                ps = psS.tile([P, 512], F32, tag="s", name=f"kt{n}")
                for d in range(8):
                    nc.tensor.matmul(ps[:], lhsT=(wk[:, d, :]),
                                     rhs=(xT[:, d, n * 512:(n + 1) * 512]),
                                     start=(d == 0), stop=(d == 7))
                nc.vector.tensor_copy(out=KT[:, n * 512:(n + 1) * 512], in_=ps[:])
                if KSCORE == "dr":
                    emit_kt8(n)

            QT = pp.tile([P, 2, NTOK], BF16, tag="QT")
            # duplicated-head layout: head hh twice along partitions, so score
            # matmuls use full 128-row tiles (DR matmuls break on HW when
            # mixed with 64-row tiles at partition 64 + psum slot reuse)
            QT4 = pp.tile([P, 4, NTOK], BF16, tag="QT4")
            # fp8 DoubleRow layouts: contraction (p, i) with the head dim
            # replicated in the i slot (scores come out doubled, same as the
            # bf16 dup path; exp scale folds the factor away).  fp8
            # conversions run on the otherwise-idle Pool engine (SBUF only).
            KT8 = pp.tile([64, 2, NTOK], F8, tag="KT8")
            QT8 = pp.tile([64, 4, 2, NTOK], F8, tag="QT8")
            QTH = pp.tile([64, 2, NTOK], BF16, tag="QTH")

            def emit_kt8(n):
                for i in range(2):
                    nc.gpsimd.tensor_copy(
                        out=KT8[:, i, n * 512:(n + 1) * 512],
                        in_=KT[0:64, n * 512:(n + 1) * 512])

            def emit_qt8(m, h, n):
                hh = 2 * m + h
                sl = slice(n * 512, (n + 1) * 512)
                if h == 0:
                    srcq = QT[0:64, m, sl]
                else:
                    nc.sync.dma_start(QTH[:, m, sl], QT[64:128, m, sl])
                    srcq = QTH[:, m, sl]
                for i in range(2):
                    nc.gpsimd.tensor_copy(out=QT8[:, hh, i, sl], in_=srcq)

            def emit_qt4(m, h, n):
                if KSCORE == "dr":
                    emit_qt8(m, h, n)
                    return
                hh = 2 * m + h
                src_ap = QT[h * DH:(h + 1) * DH, m, n * 512:(n + 1) * 512]
                eng = nc.gpsimd if (n + h) % 2 else nc.sync
                eng.dma_start(QT4[0:64, hh, n * 512:(n + 1) * 512], src_ap)
                eng.dma_start(QT4[64:128, hh, n * 512:(n + 1) * 512], src_ap)

            def emit_qt(m, n):
                ps = psS.tile([P, 512], F32, tag="s", name=f"qt{m}_{n}")
                for d in range(8):
                    nc.tensor.matmul(ps[:],
                                     lhsT=(wq[:, d, m * P:(m + 1) * P]),
                                     rhs=(xT[:, d, n * 512:(n + 1) * 512]),
                                     start=(d == 0), stop=(d == 7))
                nc.vector.tensor_copy(out=QT[:, m, n * 512:(n + 1) * 512],
                                      in_=ps[:])

            Vb = pp.tile([P, 16, DH], BF16, tag="Vb")

            def emit_v(t):
                ps = psS.tile([P, DH], F32, tag="s", name=f"v{t}")
                for d in range(8):
                    nc.tensor.matmul(ps[:],
                                     lhsT=(xT[:, d, t * P:(t + 1) * P]),
                                     rhs=(wv[:, d, :]),
                                     start=(d == 0), stop=(d == 7))
                nc.vector.tensor_copy(out=Vb[:, t, 0:DH], in_=ps[:])

            emit_kt(0)
            for n in (0, 1):
                emit_qt(0, n)
                for h in range(2):
                    emit_qt4(0, h, n)

            # ---- attention block state --------------------------------
            OTn = [op_.tile([P, NTOK], BF16, tag="otn", name=f"otn{m}")
                   for m in range(2)]

            on2_tiles = {}  # (qc, m, qch) -> On2 tile

            def emit_norm(od, den_sb, qc, m, h, split=False):
                rcp = wp.tile([P, 8], F32, tag="rcp", bufs=4)
                nc.vector.reciprocal(rcp[:], den_sb[:])
                for qch in range(8):
                    key = (qc, m, qch)
                    if key not in on2_tiles:
                        on2_tiles[key] = wp.tile(
                            [P, 2, DH], BF16, tag="on2", bufs=24,
                            name=f"on2_{qc}_{m}_{qch}")
                    dst = on2_tiles[key][:, h, :]
                    srcp = od[:, qch * DH:(qch + 1) * DH]
                    if split and qch % 2 == 1:
                        nc.scalar.activation(
                            dst, srcp, mybir.ActivationFunctionType.Copy,
                            scale=rcp[:, qch:qch + 1])
                    else:
                        nc.vector.tensor_scalar_mul(dst, srcp,
                                                    rcp[:, qch:qch + 1])

            def emit_transpose(qc, m, qch, copy_eng="dve"):
                on2 = on2_tiles.pop((qc, m, qch))
                ot = psS.tile([P, P], BF16, tag="s", name=f"ot{qc}{m}{qch}")
                nc.tensor.matmul(ot[:], lhsT=on2[:, :, :], rhs=identT[:],
                                 is_transpose=True)
                dst = OTn[m][:, qc * TQ + qch * P: qc * TQ + (qch + 1) * P]
                if copy_eng == "act":
                    nc.scalar.activation(dst, ot[:],
                                         mybir.ActivationFunctionType.Copy)
                else:
                    nc.vector.tensor_copy(out=dst, in_=ot[:])

            of_tiles = {}

            def emit_proj_half(t, n2, copy_eng="dve"):
                if t not in of_tiles:
                    of_tiles[t] = wp.tile([P, DIM], BF16, tag="of", bufs=10,
                                          name=f"of{t}")
                of = of_tiles[t]
                pf = psS.tile([P, 512], F32, tag="s", name=f"pf{t}_{n2}")
                for mm in range(2):
                    nc.tensor.matmul(
                        pf[:],
                        lhsT=(OTn[mm][:, t * P:(t + 1) * P]),
                        rhs=(wo2[:, mm, n2 * 512:(n2 + 1) * 512]),
                        start=(mm == 0), stop=(mm == 1))
                dst = of[:, n2 * 512:(n2 + 1) * 512]
                if copy_eng == "act":
                    nc.scalar.activation(dst, pf[:],
                                         mybir.ActivationFunctionType.Copy)
                else:
                    nc.vector.tensor_copy(out=dst, in_=pf[:])
                if n2 == 1:
                    of_tiles.pop(t)
                    nc.sync.dma_start(out_d[t * P:(t + 1) * P, :], of[:])

            def emit_proj(t, copy_eng="dve"):
                for n2 in range(2):
                    eng = copy_eng
                    if copy_eng == "both":
                        eng = "dve" if n2 == 0 else "act"
                    emit_proj_half(t, n2, eng)

            # interleave hooks: (block_idx, kc) -> thunks emitted at the top
            # of that key iteration.  block_idx = qc*4 + m*2 + h.
            hooks = {}

            def add_hook(bi, kc, fn):
                hooks.setdefault((bi, kc), []).append(fn)

            # KT blocks just-in-time inside block 0; QT blocks spread over
            # the whole schedule at their first-use points so no single
            # act-paced block carries more than ~2us of projection work
            add_hook(0, 2, lambda: emit_kt(1))
            add_hook(0, 6, lambda: emit_kt(2))
            add_hook(0, 10, lambda: emit_kt(3))

            def qt_and_dup(m, n):
                emit_qt(m, n)
                for h in range(2):
                    emit_qt4(m, h, n)

            add_hook(1, 2, lambda: qt_and_dup(1, 0))
            add_hook(1, 8, lambda: qt_and_dup(1, 1))
            add_hook(2, 2, lambda: qt_and_dup(0, 2))
            add_hook(3, 2, lambda: qt_and_dup(0, 3))
            add_hook(4, 2, lambda: qt_and_dup(1, 2))
            add_hook(5, 2, lambda: qt_and_dup(1, 3))
            if KSTAGE >= 5:
                # transposes for pair (qc, m) run two blocks later
                for qch in range(8):
                    add_hook(2, 8 + (qch * 7) // 8,
                             lambda qch=qch: emit_transpose(0, 0, qch))
                    add_hook(4, 8 + (qch * 7) // 8,
                             lambda qch=qch: emit_transpose(0, 1, qch))
                    add_hook(6, 8 + (qch * 7) // 8,
                             lambda qch=qch: emit_transpose(1, 0, qch))
                # qc0 output rows while qc1 attention runs (half-proj per
                # slot to limit psum-ring contention)
                for i in range(8):
                    add_hook(5, 2 * i + 1,
                             lambda t=i: emit_proj_half(t, 0))
                    add_hook(6, 2 * i + 1,
                             lambda t=i: emit_proj_half(t, 1))

            kps = [None] * 16  # SBUF keep tiles of the current qc
            pending = []       # deferred (od, qc, m, h) normalizations
            gpipe = []         # cross-block PV pipeline (thunks)

            if KSTAGE < 2:
                # projections only; pad the rest with a dummy output
                for n in (1, 2, 3):
                    emit_kt(n)
                for m in range(2):
                    for n in range(4):
                        if (m, n) not in ((0, 0), (0, 1)):
                            emit_qt(m, n)
                for t in range(16):
                    emit_v(t)
                dummy = wp.tile([P, DIM], BF16, tag="of", bufs=3, name="dummy")
                nc.vector.memset(dummy[:], 0.0)
                for t in range(16):
                    nc.gpsimd.dma_start(out_d[t * P:(t + 1) * P, :], dummy[:])
                hooks.clear()
            for qc in range(2 if KSTAGE >= 2 else 0):
                for m in range(2):
                    for h in range(2):
                        bi = qc * 4 + m * 2 + h
                        od = psO.tile([P, 512], F32, tag="od",
                                      name=f"od{bi}")
                        den_sb = wp.tile([P, 8], F32, tag="den", bufs=4,
                                         name=f"den{bi}")
                        dpair = [None]

                        def emit_pv(kc, ee, od=od, den_sb=den_sb,
                                    dpair=dpair, bi=bi):
                            if kc % 2 == 0:
                                dpair[0] = psS.tile([P, 8], F32, tag="s",
                                                    name=f"dp{bi}_{kc}")
                            dp = dpair[0]
                            for qch in range(8):
                                first = (kc == 0 and qch == 0)
                                last = (kc == 15 and qch == 7)
                                lhsT = ee[:, qch * P:(qch + 1) * P]
                                nc.tensor.matmul(
                                    od[:, qch * DH:(qch + 1) * DH],
                                    lhsT=lhsT, rhs=Vb[:, kc, :],
                                    start=first, stop=last,
                                    skip_group_check=True)
                                nc.tensor.matmul(
                                    dp[:, qch:qch + 1],
                                    lhsT=lhsT, rhs=ones[:],
                                    start=(kc % 2 == 0 and qch == 0),
                                    stop=(kc % 2 == 1 and qch == 7),
                                    skip_group_check=True)
                            if kc % 2 == 1:
                                if kc == 1:
                                    nc.vector.tensor_copy(out=den_sb[:],
                                                          in_=dp[:])
                                else:
                                    nc.vector.tensor_add(den_sb[:],
                                                         den_sb[:], dp[:])

                        def fetch_kp(kc, qc=qc):
                            if KMASK == "dr":
                                kp = wp.tile([64, 2, TQ], F8, tag="kp",
                                             bufs=18, name=f"kp{qc}_{kc}")
                                nc.sync.dma_start(
                                    kp[:],
                                    km_d[kc * 64:(kc + 1) * 64, :,
                                         qc * TQ:(qc + 1) * TQ])
                            else:
                                kp = wp.tile([P, TQ], F8, tag="kp",
                                             bufs=18, name=f"kp{qc}_{kc}")
                                nc.sync.dma_start(
                                    kp[:],
                                    km_d.ap().rearrange("p i q -> (p i) q")
                                    [kc * P:(kc + 1) * P,
                                     qc * TQ:(qc + 1) * TQ])
                            kps[kc] = kp

                        if m == 0 and h == 0:
                            fetch_kp(0)
                            fetch_kp(1)
                        for kc in range(16):
                            if bi == 0 and h == 0:
                                emit_v(kc)
                            if kc == 7 and pending:
                                if KSTAGE >= 4:
                                    for args in pending:
                                        emit_norm(*args)
                                pending.clear()
                            for fn in hooks.get((bi, kc), ()):
                                fn()
                            if m == 0 and h == 0 and kc + 2 < 16:
                                fetch_kp(kc + 2)
                            ss = psS.tile([P, TQ], F32, tag="s")
                            nomask = (KMASK == "none")
                            for qh in range(2):
                                if KSCORE == "dr":
                                    nc.tensor.matmul(
                                        ss[:, qh * 512:(qh + 1) * 512],
                                        lhsT=(KT8[:, :, kc * P:(kc + 1) * P]),
                                        rhs=(QT8[:, 2 * m + h, :,
                                                 qc * TQ + qh * 512:
                                                 qc * TQ + (qh + 1) * 512]),
                                        start=True, stop=nomask,
                                        perf_mode=DRMODE)
                                else:
                                    nc.tensor.matmul(
                                        ss[:, qh * 512:(qh + 1) * 512],
                                        lhsT=(KT[:, kc * P:(kc + 1) * P]),
                                        rhs=(QT4[:, 2 * m + h,
                                                 qc * TQ + qh * 512:
                                                 qc * TQ + (qh + 1) * 512]),
                                        start=True, stop=nomask)
                            for qh in range(2 if not nomask else 0):
                                if KMASK == "dr":
                                    nc.tensor.matmul(
                                        ss[:, qh * 512:(qh + 1) * 512],
                                        lhsT=ident2[:],
                                        rhs=kps[kc][:, :, qh * 512:(qh + 1) * 512],
                                        start=False, stop=True,
                                        perf_mode=DRMODE)
                                else:
                                    nc.tensor.matmul(
                                        ss[:, qh * 512:(qh + 1) * 512],
                                        lhsT=identM[:],
                                        rhs=kps[kc][:, qh * 512:(qh + 1) * 512],
                                        start=False, stop=True)
                            ee = wp.tile([P, TQ], BF16, tag="ee", bufs=10)
                            # scores are doubled (K and Q both duplicated
                            # across the 128 contraction rows)
                            nc.scalar.activation(ee[:], ss[:], EXP,
                                                 scale=SCALE / 2)
                            gpipe.append(
                                lambda kc=kc, ee=ee, f=emit_pv: f(kc, ee))
                            depth = 7 if not (bi == 7 and kc >= 8) else 2
                            if KSTAGE >= 3 and len(gpipe) > depth:
                                gpipe.pop(0)()
                                if bi == 7 and kc >= 10 and gpipe and \
                                        len(gpipe) > depth:
                                    gpipe.pop(0)()
                        pending.append((od, den_sb, qc, m, h))

            if KSTAGE >= 3:
                for f in gpipe:
                    f()
            gpipe.clear()
            # tail: final normalization, last pair's transposes, qc1 rows,
            # DVE/Act split so neither serializes the drain
            if KSTAGE >= 4:
                for args in pending:
                    emit_norm(*args, split=True)
            pending.clear()
            if KSTAGE >= 5:
                for qch in range(8):
                    emit_transpose(1, 1, qch,
                                   copy_eng="act" if qch % 2 else "dve")
                    emit_proj(8 + qch, copy_eng="both")
            elif KSTAGE >= 2:
                dummy = wp.tile([P, DIM], BF16, tag="of", bufs=3, name="dummy")
                nc.vector.memset(dummy[:], 0.0)
                for t in range(16):
                    nc.gpsimd.dma_start(out_d[t * P:(t + 1) * P, :], dummy[:])

    nc.compile()
    return nc


_NC_CACHE = None
_LAST_PARTS = None


def _assemble(parts, bo):
    out = np.stack([parts[0] + parts[1] + parts[2] + parts[3],
                    parts[4] + parts[5] + parts[6] + parts[7]])
    return (out + bo[None, None, :]).astype(np.float32)


def _get_nc():
    global _NC_CACHE
    if _NC_CACHE is None:
        _NC_CACHE = _build_kernel()
    return _NC_CACHE


_IDENT2 = None
_IDENTM = np.eye(P, dtype=np.float32).astype(ml_dtypes.float8_e4m3)


def _host_consts():
    global _IDENT2
    if _IDENT2 is None:
        i2 = np.zeros((64, 2, P), dtype=np.float32)
        for p in range(64):
            for i in range(2):
                i2[p, i, 2 * p + i] = 1.0
        _IDENT2 = i2.astype(ml_dtypes.float8_e4m3)
    identT = np.eye(P, dtype=np.float32).astype(ml_dtypes.bfloat16)
    return _IDENT2, identT


def kernel(x, mask, Wq, Wk, Wv, Wo, bo, _run_kwargs=None):
    x = np.asarray(x, dtype=np.float32)
    mask = np.asarray(mask).astype(bool)
    Wq = np.asarray(Wq, dtype=np.float32)
    Wk = np.asarray(Wk, dtype=np.float32)
    Wv = np.asarray(Wv, dtype=np.float32)
    Wo = np.asarray(Wo, dtype=np.float32)
    bo = np.asarray(bo, dtype=np.float32)

    nc = _get_nc()

    ident2, identT = _host_consts()
    # ss tile is S^T [key, query]; reference masks where mask[query, key].
    keepM2 = np.ascontiguousarray(
        -240.0 * mask.T.astype(np.float32)).astype(
        ml_dtypes.float8_e4m3).reshape(NTOK // 2, 2, NTOK)

    in_maps = []
    for c in range(N_CORES):
        b, j = c // 4, c % 4
        in_maps.append({
            "xT": np.ascontiguousarray(x[b].T).astype(ml_dtypes.bfloat16),
            "keepM2": keepM2,
            "ident2": ident2,
            "identT": identT,
            "identM": _IDENTM,
            "wq": np.ascontiguousarray(
                Wq[:, j * 256:(j + 1) * 256]).astype(ml_dtypes.bfloat16),
            "wk2": np.ascontiguousarray(
                np.concatenate([Wk[:, j * DH:(j + 1) * DH]] * 2,
                               axis=1)).astype(ml_dtypes.bfloat16),
            "wv": np.ascontiguousarray(
                Wv[:, j * DH:(j + 1) * DH]).astype(ml_dtypes.bfloat16),
            "wo": np.ascontiguousarray(
                Wo[j * 256:(j + 1) * 256, :]).astype(ml_dtypes.bfloat16),
        })

    res = run_bass_kernel_spmd(nc, in_maps, list(range(N_CORES)),
                               **(_run_kwargs or {}))
    parts = [res.results[c]["out"].astype(np.float32) for c in range(N_CORES)]
    global _LAST_PARTS
    _LAST_PARTS = parts
    out = _assemble(parts, bo)
    if _run_kwargs:
        kernel.last_results = res
    return out


if __name__ == "__main__":
    pass
